# revision 3
# baseline (speedup 1.0000x reference)
"""Trainium2 Bass kernel for strided-conv-as-linear (nn_ConvNd_60851096649851).

Computation (see reference): x [B,1024,1024] f32, weight [16,256] f32.
16x16 windows at stride 8 -> 127x127 patches; per patch y = W @ flat(window)
(16 outputs), reshaped to a 4x4 tile of the [B,508,508] output.

Strategy: data-parallel over batch (4 images per core, 8 cores). Per core:
64-row strips (7 patch-rows each; 19 strips cover nH=127, last anchored at
H-64). Banded bf16 weights contract the 16 kh taps along the K=64 partition
dim; the 16 kw taps are accumulating matmuls over stride-8 column APs of a
phase-major x layout. All 4 images are batched in the moving dim
(N = 4*128 = 512 = one PSUM bank).

The PE 128x128 array is row-tiled 64x128 (tile_position (0,0) / (64,0)):
two strips stream CONCURRENTLY on the two independent row-tiles (SBUF
partitions 0-63 / 64-127), halving matmul wall time vs a single 128-row
banded scheme. Outputs leave PSUM as bf16 (DVE copy) and DMA out densely.
Host casts x to bf16, pads the row width by 8, phase-deinterleaves rows,
and unscrambles the device output into [B,508,508].
"""

import os
import sys

sys.path.insert(0, "/opt/trn_rl_repo")
os.environ.setdefault("JAX_PLATFORMS", "cpu")

import numpy as np

import concourse.bass as bass  # noqa: F401
import concourse.tile as tile
from concourse import bacc, mybir
from concourse.bass_utils import run_bass_kernel_spmd

N_CORES = 8
KH = KW = 16
STRIDE = 8
D0 = D1 = 4  # per-patch output tile
OC = 16  # outputs per patch = D0*D1
SK = 64  # strip rows = K per PE row-tile
PS = 7  # full patch-rows per strip
M = PS * OC  # 112 outputs per strip

_MM_DTYPE = mybir.dt.bfloat16
_BF16 = mybir.dt.np(mybir.dt.bfloat16)


def _tile_starts(H):
    """Start rows of 64-row strips covering all patch rows."""
    nH = (H - KH) // STRIDE + 1
    n_strips = (nH + PS - 1) // PS
    starts = [min(STRIDE * PS * s, H - SK) for s in range(n_strips)]
    return starts, nH


def build_wband(weight):
    """Banded weights: [128, KW*M] bf16, identical on both partition halves.

    wb[8*il+kh, kw, il*OC+o] = W[o, kh*16+kw], il in [0,7), kh in [0,16).
    """
    W4 = np.asarray(weight, np.float32).reshape(OC, KH, KW)
    wb = np.zeros((SK, KW, M), np.float32)
    for il in range(PS):
        for kh in range(KH):
            wb[8 * il + kh, :, il * OC : (il + 1) * OC] = W4[:, kh, :].T
    full = np.concatenate([wb, wb], axis=0)  # both row-tiles
    return np.ascontiguousarray(full.reshape(128, KW * M)).astype(_BF16)


def build_nc(n_img, H, W):
    """Build the per-core Bass program. Returns compiled nc."""
    starts, nH = _tile_starts(H)
    n_strips = len(starts)
    nW = (W - KW) // STRIDE + 1
    nWp = ((nW + 3) // 4) * 4  # pad j to multiple of 4 (128)
    NM = W // STRIDE + 1  # phase-deinterleaved positions per row (129)
    WS = STRIDE * NM  # host-padded row width (phase-major layout)
    NF = n_img * nWp  # moving free size per matmul (512)
    n_slots = (n_strips + 1) // 2

    nc = bacc.Bacc(
        "TRN2", target_bir_lowering=False, debug=False, num_devices=N_CORES
    )
    f32 = mybir.dt.float32
    x_d = nc.dram_tensor("x", [n_img, H, WS], _MM_DTYPE, kind="ExternalInput").ap()
    wb_d = nc.dram_tensor(
        "wb", [128, KW * M], _MM_DTYPE, kind="ExternalInput"
    ).ap()
    out_d = nc.dram_tensor(
        "out", [n_strips, M, NF], _MM_DTYPE, kind="ExternalOutput"
    ).ap()

    with tile.TileContext(nc) as tc:
        with (
            tc.tile_pool(name="wbp", bufs=1) as wbp,
            tc.tile_pool(name="xp", bufs=3) as xp,
            tc.tile_pool(name="psp", bufs=4, space="PSUM") as psp,
            tc.tile_pool(name="op", bufs=4) as op,
        ):
            wb_sb = wbp.tile([128, KW * M], _MM_DTYPE)
            nc.sync.dma_start(wb_sb[:], wb_d[:])

            for slot in range(n_slots):
                strips = [2 * slot]
                if 2 * slot + 1 < n_strips:
                    strips.append(2 * slot + 1)
                xt = xp.tile(
                    [128, n_img * WS], _MM_DTYPE, name=f"xt{slot}", tag="xt"
                )
                for half, s in enumerate(strips):
                    r0 = starts[s]
                    for b in range(n_img):
                        nc.gpsimd.dma_start(
                            xt[
                                SK * half : SK * half + SK,
                                b * WS : (b + 1) * WS,
                            ],
                            x_d[b, r0 : r0 + SK, :],
                        )
                xt4 = xt.rearrange(
                    "p (b f m) -> p b f m", b=n_img, f=STRIDE, m=NM
                )
                ps = [
                    psp.tile([128, NF], f32, name=f"ps_{slot}_{h}", tag="ps")
                    for h in range(len(strips))
                ]
                for kw in range(KW):
                    phi = kw % STRIDE
                    m0 = kw // STRIDE
                    for half in range(len(strips)):
                        p0 = SK * half
                        rhs = xt4[p0 : p0 + SK, :, phi, m0 : m0 + nWp]
                        lhsT = wb_sb[p0 : p0 + SK, kw * M : (kw + 1) * M]
                        nc.tensor.matmul(
                            ps[half][0:M, :],
                            lhsT,
                            rhs,
                            start=(kw == 0),
                            stop=(kw == KW - 1),
                            tile_position=(p0, 0),
                        )
                for half, s in enumerate(strips):
                    ob = op.tile([128, NF], _MM_DTYPE, name="ob")
                    nc.vector.tensor_copy(ob[0:M, :], ps[half][0:M, :])
                    nc.sync.dma_start(out_d[s], ob[0:M, :])
    nc.compile()
    return nc, starts, nH, nW, n_slots


def unscramble(dev_out, starts, nH, nW, n_img):
    """dev_out [n_strips, M, n_img*nWp] bf16 -> [n_img, nH*4, nW*4] f32."""
    n_strips = dev_out.shape[0]
    nWp = ((nW + 3) // 4) * 4
    # [s, il, d0, d1, b, j]
    dev = (
        np.asarray(dev_out, np.float32)
        .reshape(n_strips, PS, D0, D1, n_img, nWp)[..., :nW]
    )
    out5 = np.empty((n_img, nH, D0, nW, D1), np.float32)
    for s in range(n_strips):
        i0 = starts[s] // STRIDE
        n_il = min(PS, nH - i0)
        # dev[s, il, d0, d1, b, j] -> out5[b, i0+il, d0, j, d1]
        out5[:, i0 : i0 + n_il] = dev[s, :n_il].transpose(3, 0, 1, 4, 2)
    return out5.reshape(n_img, nH * D0, nW * D1)


def prep_x(x, n_img):
    """Cast to bf16, pad width by STRIDE, and phase-deinterleave each row:
    xd[b, r, phi, m] = x_pad[b, r, STRIDE*m + phi], flattened to
    [B, H, STRIDE*NM]. This makes each kw tap's matmul moving operand a
    contiguous SBUF slice. Split per core."""
    B, H, W = x.shape
    NM = W // STRIDE + 1
    xb = np.zeros((B, H, STRIDE * NM), _BF16)
    xb[:, :, :W] = x.astype(_BF16)
    xd = np.ascontiguousarray(
        xb.reshape(B, H, NM, STRIDE).transpose(0, 1, 3, 2)
    ).reshape(B, H, STRIDE * NM)
    return [
        np.ascontiguousarray(xd[c * n_img : (c + 1) * n_img])
        for c in range(N_CORES)
    ]


_CACHE = {}


def _get_nc(n_img, H, W):
    key = (n_img, H, W)
    if key not in _CACHE:
        _CACHE[key] = build_nc(n_img, H, W)
    return _CACHE[key]


def kernel(x, weight):
    x = np.asarray(x, np.float32)
    weight = np.asarray(weight, np.float32)
    B, H, W = x.shape
    assert B % N_CORES == 0
    n_img = B // N_CORES
    nc, starts, nH, nW, n_slots = _get_nc(n_img, H, W)
    wb = build_wband(weight)
    x_shards = prep_x(x, n_img)
    in_maps = [{"x": x_shards[c], "wb": wb} for c in range(N_CORES)]
    results = run_bass_kernel_spmd(
        nc, in_maps, core_ids=list(range(N_CORES))
    ).results
    shards = [
        unscramble(results[c]["out"], starts, nH, nW, n_img)
        for c in range(N_CORES)
    ]
    return np.concatenate(shards, axis=0)
